# revision 10
# baseline (speedup 1.0000x reference)
"""GAT message-passing kernel for Trainium2 (8 NeuronCores, Bass/Tile).

Strategy (edge-parallel, dst-block partitioning): the model output
y = elu(sum(xo[0] * xo[1:item_len], 1)) depends only on output rows
0..item_len-1, so only edges with dst < item_len contribute (~33K of
3.2M edges). Core k owns dst rows [128k, 128k+128); every core also
processes the dst==0 edges so xo[0] is available locally.

Gather: per-128-row indirect DMA on the GpSimd SWDGE queue costs
~994 ns fixed + ~0.7 ns/row (measured 1090 ns/tile + ~309 ns dispatch
gap) — the 34-tile stream (~47 us) is the kernel's critical path.
The SWDGE ucode honors only 128 offsets per instruction (extra index
columns are misread as a larger element size), so batching beyond 128
rows/instruction is not possible; everything else is arranged to hide
under the stream:

  tensor:  ad_e column = S_t^T @ a_d        (host-shipped one-hot S)
  dve/act: xbf = bf16(x_src)                (cast, engines alternated)
  dve:     a_s = rowsum(xbf * w_s)          (bf16 stt with accumulate)
  batched: p = exp(leaky(a_s + a_d))        (per tile-group)
  dve:     S_p = p * S_t                    (scale one-hot rows by p)
  tensor:  acc += S_p^T @ [xbf | 1]         (bf16 matmul, f32 PSUM)

Scaling the 128-wide one-hot block by p (instead of scaling the
256-wide gathered row) plus a constant ones-column for z cuts per-tile
DVE/ACT work ~2x, so compute never throttles the gather stream (DVE
port pressure also slows the Q7 descriptor writes). The group pipeline
is software-skewed with tapered final groups; the main accumulator is
split four ways so 3/4 of the (u@W) epilogue runs mid-stream.
out = (u@W)/z + bias; xo = elu(out); y = elu(xo_m @ xo[0]). The host
precomputes W@att_src / W@att_dst, casts W to bf16, and builds the
one-hot S / S^T blocks (index metadata only; x is never touched beyond
contiguous slicing).
"""
import math

import numpy as np
import ml_dtypes

P = 128
N_CORES = 8
NEG_SLOPE = 0.2
IND = 256
OUTD = 128
W258 = IND + 2  # xbf tile stride: 256 data + 1 ones column + 1 pad
BF16 = ml_dtypes.bfloat16
KGRP = 4  # tiles per compute group (small-op batching)

_CACHE = {}


def _build_program(n_nodes, T_b, T_d):
    import concourse.bass as bass
    import concourse.bacc as bacc
    import concourse.tile as tile
    import concourse.mybir as mybir
    from contextlib import ExitStack

    f32 = mybir.dt.float32
    bf16 = mybir.dt.bfloat16
    i32 = mybir.dt.int32
    Alu = mybir.AluOpType
    Act = mybir.ActivationFunctionType

    T_all = T_b + T_d + 1  # dst0 tiles + data tiles + self tile
    T_idx = max(T_b + T_d, 128)  # pad idx lines to 512B for SDMA line rate

    nc = bacc.Bacc(
        "TRN2", target_bir_lowering=False, debug=False, num_devices=N_CORES
    )
    x_in = nc.dram_tensor("x_in", [n_nodes, IND], f32, kind="ExternalInput").ap()
    idx_in = nc.dram_tensor("idx_in", [P, T_idx], i32, kind="ExternalInput").ap()
    xself_in = nc.dram_tensor("xself_in", [P, IND], f32, kind="ExternalInput").ap()
    x0_in = nc.dram_tensor("x0_in", [1, IND], f32, kind="ExternalInput").ap()
    s_in = nc.dram_tensor("s_in", [P, T_all * P], bf16, kind="ExternalInput").ap()
    st_in = nc.dram_tensor("st_in", [P, T_all * P], bf16, kind="ExternalInput").ap()
    ws_in = nc.dram_tensor("ws_in", [1, IND], f32, kind="ExternalInput").ap()
    wd_in = nc.dram_tensor("wd_in", [1, IND], f32, kind="ExternalInput").ap()
    w_in = nc.dram_tensor("w_in", [IND, OUTD], bf16, kind="ExternalInput").ap()
    bias_in = nc.dram_tensor("bias_in", [1, OUTD], f32, kind="ExternalInput").ap()
    ident_in = nc.dram_tensor("ident_in", [P, P], bf16, kind="ExternalInput").ap()
    y_out = nc.dram_tensor("y_out", [1, P], f32, kind="ExternalOutput").ap()

    with tile.TileContext(nc) as tc, ExitStack() as ctx:
        const = ctx.enter_context(tc.tile_pool(name="const", bufs=1))
        idxp = ctx.enter_context(tc.tile_pool(name="idx", bufs=1))
        xgp = ctx.enter_context(tc.tile_pool(name="xg", bufs=T_d))
        xgbp = ctx.enter_context(tc.tile_pool(name="xgb", bufs=max(T_b, 1)))
        xbfp = ctx.enter_context(tc.tile_pool(name="xbf", bufs=KGRP + 3))
        spp = ctx.enter_context(tc.tile_pool(name="sp", bufs=KGRP + 3))
        scrp = ctx.enter_context(tc.tile_pool(name="scr", bufs=3))
        grpp = ctx.enter_context(tc.tile_pool(name="grp", bufs=12))
        smallp = ctx.enter_context(tc.tile_pool(name="small", bufs=10))
        utp = ctx.enter_context(tc.tile_pool(name="ut", bufs=2))
        xotr = ctx.enter_context(tc.tile_pool(name="xotr", bufs=4))
        xop = ctx.enter_context(tc.tile_pool(name="xo", bufs=2))
        accsb = ctx.enter_context(tc.tile_pool(name="accsb", bufs=2))
        # PSUM banks: acc_b 1 + acc_m 3 + tp 1 + outp 2 + adp 1 = 8
        accp = ctx.enter_context(tc.tile_pool(name="acc", bufs=4, space="PSUM"))
        tpp = ctx.enter_context(tc.tile_pool(name="tp", bufs=1, space="PSUM"))
        outpp = ctx.enter_context(tc.tile_pool(name="outp", bufs=1, space="PSUM"))
        adpp = ctx.enter_context(tc.tile_pool(name="adp", bufs=1, space="PSUM"))

        # ---- index DMA first: the gather stream depends only on this ----
        idx_t = idxp.tile([P, T_idx], i32, tag="idx")
        nc.sync.dma_start(idx_t[:], idx_in[:])

        # dst==0 block tiles, then data tiles: one indirect DMA per 128 rows
        xg_b = []
        for tb in range(T_b):
            xb = xgbp.tile([P, IND], f32, tag="xgb")
            nc.gpsimd.indirect_dma_start(
                out=xb[:],
                out_offset=None,
                in_=x_in[:],
                in_offset=bass.IndirectOffsetOnAxis(
                    ap=idx_t[:, tb : tb + 1], axis=0
                ),
            )
            xg_b.append(xb)
        xg_d = []
        for t in range(T_d):
            xg = xgp.tile([P, IND], f32, tag="xg")
            nc.gpsimd.indirect_dma_start(
                out=xg[:],
                out_offset=None,
                in_=x_in[:],
                in_offset=bass.IndirectOffsetOnAxis(
                    ap=idx_t[:, T_b + t : T_b + t + 1], axis=0
                ),
            )
            xg_d.append(xg)

        # ---- remaining input DMAs (small + early-needed first) ----
        xself_t = const.tile([P, IND], f32, tag="xself")
        nc.sync.dma_start(xself_t[:], xself_in[:])
        x0_t = const.tile([1, IND], f32, tag="x0")
        nc.sync.dma_start(x0_t[:], x0_in[:])
        ws_t = const.tile([1, IND], f32, tag="ws_t")
        nc.sync.dma_start(ws_t[:], ws_in[:])
        wd_t = const.tile([1, IND], f32, tag="wd_t")
        nc.sync.dma_start(wd_t[:], wd_in[:])
        # st needed first (stage-A ad matmuls), s shortly after; parallel queues
        st_t = const.tile([P, T_all * P], bf16, tag="st_t")
        nc.sync.dma_start(st_t[:], st_in[:])
        s_t = const.tile([P, T_all * P], bf16, tag="s_t")
        nc.scalar.dma_start(s_t[:], s_in[:])
        W0 = const.tile([P, OUTD], bf16, tag="W0")
        nc.scalar.dma_start(W0[:], w_in[0:P, :])
        W1 = const.tile([P, OUTD], bf16, tag="W1")
        nc.scalar.dma_start(W1[:], w_in[P : 2 * P, :])
        bias_t = const.tile([1, OUTD], f32, tag="bias")
        nc.scalar.dma_start(bias_t[:], bias_in[:])
        ident = const.tile([P, P], bf16, tag="ident")
        nc.scalar.dma_start(ident[:], ident_in[:])

        ones_f = const.tile([1, P], f32, tag="ones_f")
        nc.vector.memset(ones_f[:], 1.0)
        ones_b = const.tile([1, P], bf16, tag="ones_b")
        nc.vector.memset(ones_b[:], 1.0)

        # ---- prologue: broadcast weights / attention rows (f32) ----
        def bcast_f32(row_ap, width):
            bp = tpp.tile([P, IND], f32, tag="tp")
            nc.tensor.matmul(
                bp[:, :width], lhsT=ones_f[:], rhs=row_ap, start=True, stop=True,
                skip_group_check=True,
            )
            return bp

        wsp = bcast_f32(ws_t[:], IND)
        wsb = const.tile([P, IND], bf16, tag="wsb")
        nc.vector.tensor_copy(wsb[:], wsp[:, :IND])
        wdp = bcast_f32(wd_t[:], IND)
        wdb = const.tile([P, IND], f32, tag="wdb")
        nc.vector.tensor_copy(wdb[:], wdp[:, :IND])
        bp = bcast_f32(bias_t[:], OUTD)
        bias_b = const.tile([P, OUTD], f32, tag="bias_b")
        nc.vector.tensor_copy(bias_b[:], bp[:, :OUTD])

        # a_d per local row (bf16 column feeds the per-tile S^T matmuls)
        scr0 = scrp.tile([P, IND], f32, tag="scrf")
        ad_col = smallp.tile([P, 1], f32, tag="ad_col")
        nc.vector.scalar_tensor_tensor(
            out=scr0[:], in0=xself_t[:], scalar=0.0, in1=wdb[:],
            op0=Alu.bypass, op1=Alu.mult, accum_out=ad_col[:],
        )
        ad_bf = const.tile([P, 1], bf16, tag="ad_bf")
        nc.vector.tensor_copy(ad_bf[:], ad_col[:])

        # a_d[0] (node 0), broadcast to all partitions (for the dst==0 block)
        scr1 = scrp.tile([P, IND], f32, tag="scrf")
        ad0_f = smallp.tile([1, 1], f32, tag="ad0_f")
        nc.vector.scalar_tensor_tensor(
            out=scr1[0:1, :], in0=x0_t[:], scalar=0.0, in1=wdb[0:1, :],
            op0=Alu.bypass, op1=Alu.mult, accum_out=ad0_f[:],
        )
        ad0_bf = smallp.tile([1, 1], bf16, tag="ad0_bf")
        nc.vector.tensor_copy(ad0_bf[:], ad0_f[:])
        ad0p = tpp.tile([P, IND], f32, tag="tp")
        nc.tensor.matmul(
            ad0p[:, 0:1], lhsT=ones_b[:], rhs=ad0_bf[:], start=True, stop=True,
            skip_group_check=True,
        )
        ad0_col = smallp.tile([P, 1], f32, tag="ad0_col")
        nc.vector.tensor_copy(ad0_col[:], ad0p[:, 0:1])

        acc_b = accp.tile([P, IND + 1], f32, tag="acc")
        NACC = 3
        acc_m = []
        for _ai in range(NACC):
            acc_mi = accp.tile([P, IND + 1], f32, tag="acc")
            acc_m.append(acc_mi)
        # single persistent ad PSUM tile; groups alternate column halves
        adp_t = adpp.tile([P, 2 * KGRP], f32, tag="adp")

        # ---- epilogue helpers ----
        def out_phase(acc, outp, first, last, tag):
            u_bf = accsb.tile([P, IND], bf16, tag="u_bf")
            nc.vector.tensor_copy(u_bf[:], acc[:, 0:IND])
            z = smallp.tile([P, 1], f32, tag=f"z{tag}")
            nc.vector.tensor_scalar_add(z[:], acc[:, IND : IND + 1], 1e-30)
            for ci in range(2):
                tp = tpp.tile([P, P], bf16, tag="tp")
                nc.tensor.transpose(tp[:], u_bf[:, ci * P : (ci + 1) * P], ident[:])
                uT = utp.tile([P, P], bf16, tag="uT")
                nc.vector.tensor_copy(uT[:], tp[:])
                nc.tensor.matmul(
                    outp[:], lhsT=uT[:], rhs=(W0 if ci == 0 else W1)[:],
                    start=(first and ci == 0), stop=(last and ci == 1),
                    skip_group_check=True,
                )
            return z

        # xo = elu((u@W)/z + bias), for the dst==0 accumulator
        def out_block(acc, tag):
            outp = outpp.tile([P, OUTD], f32, tag="outpb")
            z = out_phase(acc, outp, True, True, tag)
            rz = smallp.tile([P, 1], f32, tag=f"rz{tag}")
            nc.vector.reciprocal(rz[:], z[:])
            outn = xotr.tile([P, OUTD], f32, tag="outn")
            nc.vector.scalar_tensor_tensor(
                out=outn[:], in0=outp[:], scalar=rz[:], in1=bias_b[:],
                op0=Alu.mult, op1=Alu.add,
            )
            tneg = xotr.tile([P, OUTD], f32, tag="tneg")
            nc.vector.tensor_scalar_min(tneg[:], outn[:], 0.0)
            texp = xotr.tile([P, OUTD], f32, tag="texp")
            nc.scalar.activation(texp[:], tneg[:], Act.Exp)
            xo = xop.tile([P, OUTD], bf16, tag="xo")
            nc.vector.scalar_tensor_tensor(
                out=xo[:], in0=texp[:], scalar=-1.0, in1=outn[:],
                op0=Alu.add, op1=Alu.max,
            )
            return xo

        # ---- processing sequence over S-block indices: dst0 blocks first
        # (their epilogue feeds xo[0]), then the self tile (its direct DMA
        # lands early), then the gathered data tiles in stream order.
        # seq[pos] = S/ST block index; source AP derived from it. ----
        seq = list(range(T_b)) + [T_b + T_d] + [T_b + t for t in range(T_d)]
        NPOS = len(seq)

        def pos_src(pos):
            sb = seq[pos]
            if sb < T_b:
                return xg_b[sb][:]
            if sb == T_b + T_d:
                return xself_t[:]
            return xg_d[sb - T_b][:]

        # ---- compute groups, software-pipelined with a one-group skew so
        # the in-order engine queues never head-of-line block on a
        # not-yet-gathered tile; the endgame runs single-tile groups so the
        # post-stream dependency chain is as short as possible ----
        groups = [list(range(T_b))]
        rest = list(range(T_b, NPOS))
        taper = [2, 1, 1, 1] if len(rest) > KGRP + 6 else []
        head = len(rest) - sum(taper)
        for i in range(0, head, KGRP):
            groups.append(rest[i : min(i + KGRP, head)])
        pos = head
        for tsz in taper:
            groups.append(rest[pos : pos + tsz])
            pos += tsz
        ngroups = len(groups)

        # accumulator index by position: the final acc covers only the
        # endgame tiles so its epilogue chain after the stream is short
        c2 = T_b + max(0, NPOS - T_b - 5)
        c1 = T_b + (c2 - T_b) // 2
        bounds = [T_b, c1, c2, NPOS]

        def pos_acc(pos):
            for i in range(NACC):
                if bounds[i] <= pos < bounds[i + 1]:
                    return i
            raise AssertionError

        state = {}

        def stage_a(gi):
            poss = groups[gi]
            as_g = grpp.tile([P, KGRP], f32, tag="as")
            a0 = (gi % 2) * KGRP
            adp = adp_t[:, a0 : a0 + KGRP]
            xbfs = []
            for j, pos in enumerate(poss):
                sb = seq[pos]
                src = pos_src(pos)
                if gi > 0:
                    nc.tensor.matmul(
                        adp[:, j : j + 1], lhsT=st_t[:, sb * P : (sb + 1) * P],
                        rhs=ad_bf[:], start=True, stop=True, skip_group_check=True,
                    )
                # bf16 cast of the gathered rows + constant ones column;
                # alternate engines mid-stream (neither DVE nor ACT may
                # wall); endgame casts stay on DVE so ACT's queue is clear
                # for the leaky/exp/scale chain
                xbf = xbfp.tile([P, W258], bf16, tag="xbf")
                if pos % 3 != 0 and pos < c2:
                    nc.scalar.activation(xbf[:, 0:IND], src, Act.Copy)
                else:
                    nc.vector.tensor_copy(xbf[:, 0:IND], src)
                nc.vector.memset(xbf[:, IND : IND + 1], 1.0)
                scr = scrp.tile([P, IND], bf16, tag="scr")
                nc.vector.scalar_tensor_tensor(
                    out=scr[:], in0=xbf[:, 0:IND], scalar=0.0, in1=wsb[:],
                    op0=Alu.bypass, op1=Alu.mult,
                    accum_out=as_g[:, j : j + 1],
                )
                xbfs.append(xbf)
            state[gi] = (as_g, adp, xbfs)

        def stage_b(gi):
            poss = groups[gi]
            k = len(poss)
            as_g, adp, xbfs = state.pop(gi)
            v_g = grpp.tile([P, KGRP], f32, tag="v")
            if gi == 0:
                # dst0 block: a_d is node 0's value on every partition
                nc.vector.tensor_scalar(
                    out=v_g[:, 0:k], in0=as_g[:, 0:k], scalar1=ad0_col[:],
                    scalar2=None, op0=Alu.add,
                )
            else:
                nc.vector.tensor_tensor(
                    out=v_g[:, 0:k], in0=as_g[:, 0:k], in1=adp[:, 0:k],
                    op=Alu.add,
                )
            e_g = grpp.tile([P, KGRP], f32, tag="e")
            nc.vector.scalar_tensor_tensor(
                out=e_g[:, 0:k], in0=v_g[:, 0:k], scalar=NEG_SLOPE,
                in1=v_g[:, 0:k], op0=Alu.mult, op1=Alu.max,
            )
            p_g = grpp.tile([P, KGRP], f32, tag="p")
            nc.scalar.activation(p_g[:, 0:k], e_g[:, 0:k], Act.Exp)
            for j, pos in enumerate(poss):
                sb = seq[pos]
                s_p = spp.tile([P, P], bf16, tag="sp")
                if pos < c2:
                    nc.scalar.activation(
                        s_p[:], s_t[:, sb * P : (sb + 1) * P], Act.Copy,
                        scale=p_g[:, j : j + 1],
                    )
                else:
                    nc.vector.tensor_scalar(
                        out=s_p[:], in0=s_t[:, sb * P : (sb + 1) * P],
                        scalar1=p_g[:, j : j + 1], scalar2=None, op0=Alu.mult,
                    )
                rhs = xbfs[j][:, 0 : IND + 1]
                if gi == 0:
                    nc.tensor.matmul(
                        acc_b[:], lhsT=s_p[:], rhs=rhs,
                        start=(pos == 0), stop=(pos == T_b - 1),
                        skip_group_check=True,
                    )
                else:
                    ai = pos_acc(pos)
                    nc.tensor.matmul(
                        acc_m[ai][:], lhsT=s_p[:], rhs=rhs,
                        start=(pos == bounds[ai]),
                        stop=(pos == bounds[ai + 1] - 1),
                        skip_group_check=True,
                    )
            if gi == 0:
                # dst==0 block complete: fold its epilogue under the stream
                state["xo_b"] = out_block(acc_b, "b")

        # group index right after which accumulator ai is complete
        def acc_done_group(ai):
            last_pos = bounds[ai + 1] - 1
            for gi, poss in enumerate(groups):
                if last_pos in poss:
                    return gi
            raise AssertionError

        done_at = {acc_done_group(ai): ai for ai in range(NACC - 1)}

        outp_m = outpp.tile([P, OUTD], f32, tag="outp")
        zs = []
        for gi in range(ngroups):
            stage_a(gi)
            stage_b(gi)
            if gi in done_at:
                ai = done_at[gi]
                zs.append(out_phase(acc_m[ai], outp_m, ai == 0, False, f"m{ai}"))
        xo_b = state["xo_b"]
        zz01 = smallp.tile([P, 1], f32, tag="zz01")
        nc.vector.tensor_tensor(out=zz01[:], in0=zs[0][:], in1=zs[1][:], op=Alu.add)

        zs.append(out_phase(acc_m[NACC - 1], outp_m, False, True, f"m{NACC-1}"))
        zz = smallp.tile([P, 1], f32, tag="zz")
        nc.vector.tensor_tensor(out=zz[:], in0=zz01[:], in1=zs[2][:], op=Alu.add)
        rz = smallp.tile([P, 1], f32, tag="rzm")
        nc.vector.reciprocal(rz[:], zz[:])
        outn = xotr.tile([P, OUTD], f32, tag="outn")
        nc.vector.scalar_tensor_tensor(
            out=outn[:], in0=outp_m[:], scalar=rz[:], in1=bias_b[:],
            op0=Alu.mult, op1=Alu.add,
        )
        tneg = xotr.tile([P, OUTD], f32, tag="tneg")
        nc.vector.tensor_scalar_min(tneg[:], outn[:], 0.0)
        texp = xotr.tile([P, OUTD], f32, tag="texp")
        nc.scalar.activation(texp[:], tneg[:], Act.Exp)
        xo_m = xop.tile([P, OUTD], bf16, tag="xo")
        nc.vector.scalar_tensor_tensor(
            out=xo_m[:], in0=texp[:], scalar=-1.0, in1=outn[:],
            op0=Alu.add, op1=Alu.max,
        )

        # ---- y = elu(dot(xo[0], xo_m[j])) ----
        xo0p = tpp.tile([P, IND], f32, tag="tp")
        nc.tensor.matmul(
            xo0p[:, :OUTD], lhsT=ones_b[:], rhs=xo_b[0:1, :], start=True, stop=True,
            skip_group_check=True,
        )
        xo0s = const.tile([P, OUTD], bf16, tag="xo0s")
        nc.vector.tensor_copy(xo0s[:], xo0p[:, :OUTD])
        dscr = scrp.tile([P, OUTD], f32, tag="dscr")
        d_sb = smallp.tile([P, 1], f32, tag="d_sb")
        nc.vector.scalar_tensor_tensor(
            out=dscr[:], in0=xo_m[:], scalar=0.0, in1=xo0s[:],
            op0=Alu.bypass, op1=Alu.mult, accum_out=d_sb[:],
        )
        yneg = smallp.tile([P, 1], f32, tag="yneg")
        nc.vector.tensor_scalar_min(yneg[:], d_sb[:], 0.0)
        yexp = smallp.tile([P, 1], f32, tag="yexp")
        nc.scalar.activation(yexp[:], yneg[:], Act.Exp)
        y_bf = smallp.tile([P, 1], bf16, tag="y_bf")
        nc.vector.scalar_tensor_tensor(
            out=y_bf[:], in0=yexp[:], scalar=-1.0, in1=d_sb[:],
            op0=Alu.add, op1=Alu.max,
        )
        # write y as a contiguous [1, P] row (column DMA has a huge
        # HBM completion delay that the kernel-tail barrier waits out)
        yrp = tpp.tile([P, P], bf16, tag="tp")
        nc.tensor.transpose(yrp[:1, :], y_bf[:], ident[:])
        y_row = smallp.tile([1, P], f32, tag="y_row")
        nc.vector.tensor_copy(y_row[:], yrp[:1, :P])
        nc.sync.dma_start(y_out[:], y_row[:])

    nc.compile()
    return nc


def _get_program(n_nodes, T_b, T_d):
    key = (n_nodes, T_b, T_d)
    if key not in _CACHE:
        _CACHE[key] = _build_program(n_nodes, T_b, T_d)
    return _CACHE[key]


def _pack_cols(vals, T, pad, dtype):
    """[n] -> [P, T] column-per-tile layout (tile t, lane p) = vals[t*P+p]."""
    npad = T * P - len(vals)
    v = np.concatenate([vals, np.full(npad, pad, vals.dtype)])
    return np.ascontiguousarray(v.reshape(T, P).T).astype(dtype)


def _onehot_blocks(dst_cols):
    """dst_cols [P, T] -> (S [P, T*P], ST [P, T*P]) one-hot bf16 blocks.
    S_t[e, j] = (dst[e, t] == j); ST_t = S_t^T. dst==P rows are all-zero."""
    Pn = P
    T = dst_cols.shape[1]
    S = np.zeros((Pn, T * Pn), dtype=BF16)
    ST = np.zeros((Pn, T * Pn), dtype=BF16)
    e_idx, t_idx = np.nonzero(dst_cols < Pn)
    j_idx = dst_cols[e_idx, t_idx]
    S[e_idx, t_idx * Pn + j_idx] = 1
    ST[j_idx, t_idx * Pn + e_idx] = 1
    return np.ascontiguousarray(S), np.ascontiguousarray(ST)


def prepare(x, edge_index, W, att_src, att_dst, bias, item_len):
    """Python-side edge partitioning; returns (nc, in_maps, item_len)."""
    item_len = int(np.asarray(item_len))
    x = np.ascontiguousarray(np.asarray(x, np.float32))
    W = np.ascontiguousarray(np.asarray(W, np.float32))
    att_src = np.asarray(att_src, np.float32)
    att_dst = np.asarray(att_dst, np.float32)
    bias = np.asarray(bias, np.float32)
    n_nodes = x.shape[0]
    assert x.shape[1] == IND and W.shape == (IND, OUTD)
    assert item_len <= N_CORES * P, "kernel supports item_len <= 1024"

    src = np.asarray(edge_index[0])
    dst = np.asarray(edge_index[1])
    keep = dst < item_len
    src_f = src[keep].astype(np.int32)
    dst_f = dst[keep].astype(np.int32)

    # dst==0 block (graph edges + node-0 self loop), shared by all cores
    sel0 = dst_f == 0
    b_src = np.concatenate([src_f[sel0], np.zeros(1, np.int32)])
    T_b = max(1, math.ceil(len(b_src) / P))

    blk = dst_f // P
    order = np.argsort(blk, kind="stable")
    src_f = src_f[order]
    dst_f = dst_f[order]
    blk = blk[order]
    bounds = np.searchsorted(blk, np.arange(N_CORES + 1))
    T_d = max(
        1, max(math.ceil(int(bounds[k + 1] - bounds[k]) / P) for k in range(N_CORES))
    )

    nc = _get_program(n_nodes, T_b, T_d)

    # host weight preprocessing
    ws_r = np.ascontiguousarray((W @ att_src).astype(np.float32).reshape(1, IND))
    wd_r = np.ascontiguousarray((W @ att_dst).astype(np.float32).reshape(1, IND))
    w_bf = np.ascontiguousarray(W.astype(BF16))
    ident = np.eye(P, dtype=np.float32).astype(BF16)
    x0 = np.ascontiguousarray(x[0:1])
    bias_r = np.ascontiguousarray(bias.reshape(1, OUTD))

    b_eidx = _pack_cols(b_src, T_b, 0, np.int32)
    b_dst = _pack_cols(np.zeros(len(b_src), np.int32), T_b, P, np.int32)
    T_idx = max(T_b + T_d, 128)

    in_maps = []
    for k in range(N_CORES):
        lo, hi = bounds[k], bounds[k + 1]
        es = src_f[lo:hi]
        ed = dst_f[lo:hi] - k * P
        dst_cols = [b_dst, _pack_cols(ed, T_d, P, np.int32)]
        eidx = _pack_cols(es, T_d, 0, np.int32)
        self_dst = np.arange(P, dtype=np.int32)
        if (k + 1) * P > item_len:
            self_dst = np.where(
                np.arange(k * P, (k + 1) * P) < item_len, self_dst, P
            ).astype(np.int32)
        dst_cols.append(self_dst[:, None])
        dst_all = np.concatenate(dst_cols, axis=1)
        S, ST = _onehot_blocks(dst_all)
        xself = np.ascontiguousarray(
            x[np.minimum(np.arange(k * P, (k + 1) * P), n_nodes - 1)]
        )
        m = {
            "x_in": x,
            "idx_in": np.ascontiguousarray(np.concatenate(
                [b_eidx, eidx,
                 np.zeros((P, T_idx - T_b - T_d), np.int32)], axis=1)),
            "xself_in": xself,
            "x0_in": x0,
            "s_in": S,
            "st_in": ST,
            "ws_in": ws_r,
            "wd_in": wd_r,
            "w_in": w_bf,
            "bias_in": bias_r,
            "ident_in": ident,
        }
        in_maps.append(m)
    return nc, in_maps, item_len


def assemble(results, item_len):
    y_all = np.concatenate(
        [np.asarray(results[k]["y_out"], np.float32).ravel() for k in range(N_CORES)]
    )
    return y_all[1:item_len].astype(np.float32)


def kernel(x, edge_index, W, att_src, att_dst, bias, item_len):
    from concourse import bass_utils

    nc, in_maps, item_len = prepare(
        x, edge_index, W, att_src, att_dst, bias, item_len
    )
    res = bass_utils.run_bass_kernel_spmd(nc, in_maps, core_ids=list(range(N_CORES)))
    return assemble(res.results, item_len)


# revision 11
# speedup vs baseline: 1.0324x; 1.0324x over previous
"""GAT message-passing kernel for Trainium2 (8 NeuronCores, Bass/Tile).

Strategy (edge-parallel, dst-block partitioning): the model output
y = elu(sum(xo[0] * xo[1:item_len], 1)) depends only on output rows
0..item_len-1, so only edges with dst < item_len contribute (~33K of
3.2M edges). Core k owns dst rows [128k, 128k+128); every core also
processes the dst==0 edges so xo[0] is available locally.

Gather: per-128-row indirect DMA on the GpSimd SWDGE queue costs
~994 ns fixed + ~0.7 ns/row (measured 1090 ns/tile + ~309 ns dispatch
gap) — the 34-tile stream (~47 us) is the kernel's critical path.
The SWDGE ucode honors only 128 offsets per instruction (extra index
columns are misread as a larger element size), so batching beyond 128
rows/instruction is not possible; everything else is arranged to hide
under the stream:

  tensor:  ad_e column = S_t^T @ a_d        (host-shipped one-hot S)
  dve/act: xbf = bf16(x_src)                (cast, engines alternated)
  dve:     a_s = rowsum(xbf * w_s)          (bf16 stt with accumulate)
  batched: p = exp(leaky(a_s + a_d))        (per tile-group)
  dve:     S_p = p * S_t                    (scale one-hot rows by p)
  tensor:  acc += S_p^T @ [xbf | 1]         (bf16 matmul, f32 PSUM)

Scaling the 128-wide one-hot block by p (instead of scaling the
256-wide gathered row) plus a constant ones-column for z cuts per-tile
DVE/ACT work ~2x, so compute never throttles the gather stream (DVE
port pressure also slows the Q7 descriptor writes). The group pipeline
is software-skewed with tapered final groups; the main accumulator is
split four ways so 3/4 of the (u@W) epilogue runs mid-stream.
out = (u@W)/z + bias; xo = elu(out); y = elu(xo_m @ xo[0]). The host
precomputes W@att_src / W@att_dst, casts W to bf16, and builds the
one-hot S / S^T blocks (index metadata only; x is never touched beyond
contiguous slicing).
"""
import math

import numpy as np
import ml_dtypes

P = 128
N_CORES = 8
NEG_SLOPE = 0.2
IND = 256
OUTD = 128
W258 = IND + 2  # xbf tile stride: 256 data + 1 ones column + 1 pad
BF16 = ml_dtypes.bfloat16
KGRP = 4  # tiles per compute group (small-op batching)

_CACHE = {}


def _build_program(n_nodes, T_b, T_d):
    import concourse.bass as bass
    import concourse.bacc as bacc
    import concourse.tile as tile
    import concourse.mybir as mybir
    from contextlib import ExitStack

    f32 = mybir.dt.float32
    bf16 = mybir.dt.bfloat16
    i32 = mybir.dt.int32
    Alu = mybir.AluOpType
    Act = mybir.ActivationFunctionType

    T_all = T_b + T_d + 1  # dst0 tiles + data tiles + self tile
    T_idx = T_b + T_d  # keep idx lines narrow: gather ucode slows with offset-AP stride

    nc = bacc.Bacc(
        "TRN2", target_bir_lowering=False, debug=False, num_devices=N_CORES
    )
    x_in = nc.dram_tensor("x_in", [n_nodes, IND], f32, kind="ExternalInput").ap()
    idx_in = nc.dram_tensor("idx_in", [P, T_idx], i32, kind="ExternalInput").ap()
    xself_in = nc.dram_tensor("xself_in", [P, IND], f32, kind="ExternalInput").ap()
    x0_in = nc.dram_tensor("x0_in", [1, IND], f32, kind="ExternalInput").ap()
    s_in = nc.dram_tensor("s_in", [P, T_all * P], bf16, kind="ExternalInput").ap()
    st_in = nc.dram_tensor("st_in", [P, T_all * P], bf16, kind="ExternalInput").ap()
    ws_in = nc.dram_tensor("ws_in", [1, IND], f32, kind="ExternalInput").ap()
    wd_in = nc.dram_tensor("wd_in", [1, IND], f32, kind="ExternalInput").ap()
    w_in = nc.dram_tensor("w_in", [IND, OUTD], bf16, kind="ExternalInput").ap()
    bias_in = nc.dram_tensor("bias_in", [1, OUTD], f32, kind="ExternalInput").ap()
    ident_in = nc.dram_tensor("ident_in", [P, P], bf16, kind="ExternalInput").ap()
    y_out = nc.dram_tensor("y_out", [1, P], f32, kind="ExternalOutput").ap()

    with tile.TileContext(nc) as tc, ExitStack() as ctx:
        const = ctx.enter_context(tc.tile_pool(name="const", bufs=1))
        idxp = ctx.enter_context(tc.tile_pool(name="idx", bufs=1))
        xgp = ctx.enter_context(tc.tile_pool(name="xg", bufs=T_d))
        xgbp = ctx.enter_context(tc.tile_pool(name="xgb", bufs=max(T_b, 1)))
        xbfp = ctx.enter_context(tc.tile_pool(name="xbf", bufs=KGRP + 3))
        spp = ctx.enter_context(tc.tile_pool(name="sp", bufs=KGRP + 3))
        scrp = ctx.enter_context(tc.tile_pool(name="scr", bufs=3))
        grpp = ctx.enter_context(tc.tile_pool(name="grp", bufs=12))
        smallp = ctx.enter_context(tc.tile_pool(name="small", bufs=10))
        utp = ctx.enter_context(tc.tile_pool(name="ut", bufs=2))
        xotr = ctx.enter_context(tc.tile_pool(name="xotr", bufs=4))
        xop = ctx.enter_context(tc.tile_pool(name="xo", bufs=2))
        accsb = ctx.enter_context(tc.tile_pool(name="accsb", bufs=2))
        # PSUM banks: acc_b 1 + acc_m 3 + tp 1 + outp 2 + adp 1 = 8
        accp = ctx.enter_context(tc.tile_pool(name="acc", bufs=4, space="PSUM"))
        tpp = ctx.enter_context(tc.tile_pool(name="tp", bufs=1, space="PSUM"))
        outpp = ctx.enter_context(tc.tile_pool(name="outp", bufs=1, space="PSUM"))
        adpp = ctx.enter_context(tc.tile_pool(name="adp", bufs=1, space="PSUM"))

        # ---- index DMA first: the gather stream depends only on this ----
        idx_t = idxp.tile([P, T_idx], i32, tag="idx")
        nc.sync.dma_start(idx_t[:], idx_in[:])

        # dst==0 block tiles, then data tiles: one indirect DMA per 128 rows
        xg_b = []
        for tb in range(T_b):
            xb = xgbp.tile([P, IND], f32, tag="xgb")
            nc.gpsimd.indirect_dma_start(
                out=xb[:],
                out_offset=None,
                in_=x_in[:],
                in_offset=bass.IndirectOffsetOnAxis(
                    ap=idx_t[:, tb : tb + 1], axis=0
                ),
            )
            xg_b.append(xb)
        xg_d = []
        for t in range(T_d):
            xg = xgp.tile([P, IND], f32, tag="xg")
            nc.gpsimd.indirect_dma_start(
                out=xg[:],
                out_offset=None,
                in_=x_in[:],
                in_offset=bass.IndirectOffsetOnAxis(
                    ap=idx_t[:, T_b + t : T_b + t + 1], axis=0
                ),
            )
            xg_d.append(xg)

        # ---- remaining input DMAs (small + early-needed first) ----
        xself_t = const.tile([P, IND], f32, tag="xself")
        nc.sync.dma_start(xself_t[:], xself_in[:])
        x0_t = const.tile([1, IND], f32, tag="x0")
        nc.sync.dma_start(x0_t[:], x0_in[:])
        ws_t = const.tile([1, IND], f32, tag="ws_t")
        nc.sync.dma_start(ws_t[:], ws_in[:])
        wd_t = const.tile([1, IND], f32, tag="wd_t")
        nc.sync.dma_start(wd_t[:], wd_in[:])
        # st needed first (stage-A ad matmuls), s shortly after; parallel queues
        st_t = const.tile([P, T_all * P], bf16, tag="st_t")
        nc.sync.dma_start(st_t[:], st_in[:])
        s_t = const.tile([P, T_all * P], bf16, tag="s_t")
        nc.scalar.dma_start(s_t[:], s_in[:])
        W0 = const.tile([P, OUTD], bf16, tag="W0")
        nc.scalar.dma_start(W0[:], w_in[0:P, :])
        W1 = const.tile([P, OUTD], bf16, tag="W1")
        nc.scalar.dma_start(W1[:], w_in[P : 2 * P, :])
        bias_t = const.tile([1, OUTD], f32, tag="bias")
        nc.scalar.dma_start(bias_t[:], bias_in[:])
        ident = const.tile([P, P], bf16, tag="ident")
        nc.scalar.dma_start(ident[:], ident_in[:])

        ones_f = const.tile([1, P], f32, tag="ones_f")
        nc.vector.memset(ones_f[:], 1.0)
        ones_b = const.tile([1, P], bf16, tag="ones_b")
        nc.vector.memset(ones_b[:], 1.0)

        # ---- prologue: broadcast weights / attention rows (f32) ----
        def bcast_f32(row_ap, width):
            bp = tpp.tile([P, IND], f32, tag="tp")
            nc.tensor.matmul(
                bp[:, :width], lhsT=ones_f[:], rhs=row_ap, start=True, stop=True,
                skip_group_check=True,
            )
            return bp

        wsp = bcast_f32(ws_t[:], IND)
        wsb = const.tile([P, IND], bf16, tag="wsb")
        nc.vector.tensor_copy(wsb[:], wsp[:, :IND])
        wdp = bcast_f32(wd_t[:], IND)
        wdb = const.tile([P, IND], f32, tag="wdb")
        nc.vector.tensor_copy(wdb[:], wdp[:, :IND])
        bp = bcast_f32(bias_t[:], OUTD)
        bias_b = const.tile([P, OUTD], f32, tag="bias_b")
        nc.vector.tensor_copy(bias_b[:], bp[:, :OUTD])

        # a_d per local row (bf16 column feeds the per-tile S^T matmuls)
        scr0 = scrp.tile([P, IND], f32, tag="scrf")
        ad_col = smallp.tile([P, 1], f32, tag="ad_col")
        nc.vector.scalar_tensor_tensor(
            out=scr0[:], in0=xself_t[:], scalar=0.0, in1=wdb[:],
            op0=Alu.bypass, op1=Alu.mult, accum_out=ad_col[:],
        )
        ad_bf = const.tile([P, 1], bf16, tag="ad_bf")
        nc.vector.tensor_copy(ad_bf[:], ad_col[:])

        # a_d[0] (node 0), broadcast to all partitions (for the dst==0 block)
        scr1 = scrp.tile([P, IND], f32, tag="scrf")
        ad0_f = smallp.tile([1, 1], f32, tag="ad0_f")
        nc.vector.scalar_tensor_tensor(
            out=scr1[0:1, :], in0=x0_t[:], scalar=0.0, in1=wdb[0:1, :],
            op0=Alu.bypass, op1=Alu.mult, accum_out=ad0_f[:],
        )
        ad0_bf = smallp.tile([1, 1], bf16, tag="ad0_bf")
        nc.vector.tensor_copy(ad0_bf[:], ad0_f[:])
        ad0p = tpp.tile([P, IND], f32, tag="tp")
        nc.tensor.matmul(
            ad0p[:, 0:1], lhsT=ones_b[:], rhs=ad0_bf[:], start=True, stop=True,
            skip_group_check=True,
        )
        ad0_col = smallp.tile([P, 1], f32, tag="ad0_col")
        nc.vector.tensor_copy(ad0_col[:], ad0p[:, 0:1])

        acc_b = accp.tile([P, IND + 1], f32, tag="acc")
        NACC = 3
        acc_m = []
        for _ai in range(NACC):
            acc_mi = accp.tile([P, IND + 1], f32, tag="acc")
            acc_m.append(acc_mi)
        # single persistent ad PSUM tile; groups alternate column halves
        adp_t = adpp.tile([P, 2 * KGRP], f32, tag="adp")

        # ---- epilogue helpers ----
        def out_phase(acc, outp, first, last, tag):
            u_bf = accsb.tile([P, IND], bf16, tag="u_bf")
            nc.vector.tensor_copy(u_bf[:], acc[:, 0:IND])
            z = smallp.tile([P, 1], f32, tag=f"z{tag}")
            nc.vector.tensor_scalar_add(z[:], acc[:, IND : IND + 1], 1e-30)
            for ci in range(2):
                tp = tpp.tile([P, P], bf16, tag="tp")
                nc.tensor.transpose(tp[:], u_bf[:, ci * P : (ci + 1) * P], ident[:])
                uT = utp.tile([P, P], bf16, tag="uT")
                nc.vector.tensor_copy(uT[:], tp[:])
                nc.tensor.matmul(
                    outp[:], lhsT=uT[:], rhs=(W0 if ci == 0 else W1)[:],
                    start=(first and ci == 0), stop=(last and ci == 1),
                    skip_group_check=True,
                )
            return z

        # xo = elu((u@W)/z + bias), for the dst==0 accumulator
        def out_block(acc, tag):
            outp = outpp.tile([P, OUTD], f32, tag="outpb")
            z = out_phase(acc, outp, True, True, tag)
            rz = smallp.tile([P, 1], f32, tag=f"rz{tag}")
            nc.vector.reciprocal(rz[:], z[:])
            outn = xotr.tile([P, OUTD], f32, tag="outn")
            nc.vector.scalar_tensor_tensor(
                out=outn[:], in0=outp[:], scalar=rz[:], in1=bias_b[:],
                op0=Alu.mult, op1=Alu.add,
            )
            tneg = xotr.tile([P, OUTD], f32, tag="tneg")
            nc.vector.tensor_scalar_min(tneg[:], outn[:], 0.0)
            texp = xotr.tile([P, OUTD], f32, tag="texp")
            nc.scalar.activation(texp[:], tneg[:], Act.Exp)
            xo = xop.tile([P, OUTD], bf16, tag="xo")
            nc.vector.scalar_tensor_tensor(
                out=xo[:], in0=texp[:], scalar=-1.0, in1=outn[:],
                op0=Alu.add, op1=Alu.max,
            )
            return xo

        # ---- processing sequence over S-block indices: dst0 blocks first
        # (their epilogue feeds xo[0]), then the self tile (its direct DMA
        # lands early), then the gathered data tiles in stream order.
        # seq[pos] = S/ST block index; source AP derived from it. ----
        seq = list(range(T_b)) + [T_b + T_d] + [T_b + t for t in range(T_d)]
        NPOS = len(seq)

        def pos_src(pos):
            sb = seq[pos]
            if sb < T_b:
                return xg_b[sb][:]
            if sb == T_b + T_d:
                return xself_t[:]
            return xg_d[sb - T_b][:]

        # ---- compute groups, software-pipelined with a one-group skew so
        # the in-order engine queues never head-of-line block on a
        # not-yet-gathered tile; the endgame runs single-tile groups so the
        # post-stream dependency chain is as short as possible ----
        groups = [list(range(T_b))]
        rest = list(range(T_b, NPOS))
        taper = [2, 1, 1, 1] if len(rest) > KGRP + 6 else []
        head = len(rest) - sum(taper)
        for i in range(0, head, KGRP):
            groups.append(rest[i : min(i + KGRP, head)])
        pos = head
        for tsz in taper:
            groups.append(rest[pos : pos + tsz])
            pos += tsz
        ngroups = len(groups)

        # accumulator index by position: the final acc covers only the
        # endgame tiles so its epilogue chain after the stream is short
        c2 = T_b + max(0, NPOS - T_b - 5)
        c1 = T_b + (c2 - T_b) // 2
        bounds = [T_b, c1, c2, NPOS]

        def pos_acc(pos):
            for i in range(NACC):
                if bounds[i] <= pos < bounds[i + 1]:
                    return i
            raise AssertionError

        state = {}

        def stage_a(gi):
            poss = groups[gi]
            as_g = grpp.tile([P, KGRP], f32, tag="as")
            a0 = (gi % 2) * KGRP
            adp = adp_t[:, a0 : a0 + KGRP]
            xbfs = []
            for j, pos in enumerate(poss):
                sb = seq[pos]
                src = pos_src(pos)
                if gi > 0:
                    nc.tensor.matmul(
                        adp[:, j : j + 1], lhsT=st_t[:, sb * P : (sb + 1) * P],
                        rhs=ad_bf[:], start=True, stop=True, skip_group_check=True,
                    )
                # bf16 cast of the gathered rows + constant ones column;
                # alternate engines mid-stream (neither DVE nor ACT may
                # wall); endgame casts stay on DVE so ACT's queue is clear
                # for the leaky/exp/scale chain
                xbf = xbfp.tile([P, W258], bf16, tag="xbf")
                if pos % 3 != 0 and pos < c2:
                    nc.scalar.activation(xbf[:, 0:IND], src, Act.Copy)
                else:
                    nc.vector.tensor_copy(xbf[:, 0:IND], src)
                nc.vector.memset(xbf[:, IND : IND + 1], 1.0)
                scr = scrp.tile([P, IND], bf16, tag="scr")
                nc.vector.scalar_tensor_tensor(
                    out=scr[:], in0=xbf[:, 0:IND], scalar=0.0, in1=wsb[:],
                    op0=Alu.bypass, op1=Alu.mult,
                    accum_out=as_g[:, j : j + 1],
                )
                xbfs.append(xbf)
            state[gi] = (as_g, adp, xbfs)

        def stage_b(gi):
            poss = groups[gi]
            k = len(poss)
            as_g, adp, xbfs = state.pop(gi)
            v_g = grpp.tile([P, KGRP], f32, tag="v")
            if gi == 0:
                # dst0 block: a_d is node 0's value on every partition
                nc.vector.tensor_scalar(
                    out=v_g[:, 0:k], in0=as_g[:, 0:k], scalar1=ad0_col[:],
                    scalar2=None, op0=Alu.add,
                )
            else:
                nc.vector.tensor_tensor(
                    out=v_g[:, 0:k], in0=as_g[:, 0:k], in1=adp[:, 0:k],
                    op=Alu.add,
                )
            e_g = grpp.tile([P, KGRP], f32, tag="e")
            nc.vector.scalar_tensor_tensor(
                out=e_g[:, 0:k], in0=v_g[:, 0:k], scalar=NEG_SLOPE,
                in1=v_g[:, 0:k], op0=Alu.mult, op1=Alu.max,
            )
            p_g = grpp.tile([P, KGRP], f32, tag="p")
            nc.scalar.activation(p_g[:, 0:k], e_g[:, 0:k], Act.Exp)
            for j, pos in enumerate(poss):
                sb = seq[pos]
                s_p = spp.tile([P, P], bf16, tag="sp")
                if pos < c2:
                    nc.scalar.activation(
                        s_p[:], s_t[:, sb * P : (sb + 1) * P], Act.Copy,
                        scale=p_g[:, j : j + 1],
                    )
                else:
                    nc.vector.tensor_scalar(
                        out=s_p[:], in0=s_t[:, sb * P : (sb + 1) * P],
                        scalar1=p_g[:, j : j + 1], scalar2=None, op0=Alu.mult,
                    )
                rhs = xbfs[j][:, 0 : IND + 1]
                if gi == 0:
                    nc.tensor.matmul(
                        acc_b[:], lhsT=s_p[:], rhs=rhs,
                        start=(pos == 0), stop=(pos == T_b - 1),
                        skip_group_check=True,
                    )
                else:
                    ai = pos_acc(pos)
                    nc.tensor.matmul(
                        acc_m[ai][:], lhsT=s_p[:], rhs=rhs,
                        start=(pos == bounds[ai]),
                        stop=(pos == bounds[ai + 1] - 1),
                        skip_group_check=True,
                    )
            if gi == 0:
                # dst==0 block complete: fold its epilogue under the stream
                state["xo_b"] = out_block(acc_b, "b")

        # group index right after which accumulator ai is complete
        def acc_done_group(ai):
            last_pos = bounds[ai + 1] - 1
            for gi, poss in enumerate(groups):
                if last_pos in poss:
                    return gi
            raise AssertionError

        done_at = {acc_done_group(ai): ai for ai in range(NACC - 1)}

        outp_m = outpp.tile([P, OUTD], f32, tag="outp")
        zs = []
        for gi in range(ngroups):
            stage_a(gi)
            stage_b(gi)
            if gi in done_at:
                ai = done_at[gi]
                zs.append(out_phase(acc_m[ai], outp_m, ai == 0, False, f"m{ai}"))
        xo_b = state["xo_b"]
        zz01 = smallp.tile([P, 1], f32, tag="zz01")
        nc.vector.tensor_tensor(out=zz01[:], in0=zs[0][:], in1=zs[1][:], op=Alu.add)

        zs.append(out_phase(acc_m[NACC - 1], outp_m, False, True, f"m{NACC-1}"))
        zz = smallp.tile([P, 1], f32, tag="zz")
        nc.vector.tensor_tensor(out=zz[:], in0=zz01[:], in1=zs[2][:], op=Alu.add)
        rz = smallp.tile([P, 1], f32, tag="rzm")
        nc.vector.reciprocal(rz[:], zz[:])
        outn = xotr.tile([P, OUTD], f32, tag="outn")
        nc.vector.scalar_tensor_tensor(
            out=outn[:], in0=outp_m[:], scalar=rz[:], in1=bias_b[:],
            op0=Alu.mult, op1=Alu.add,
        )
        tneg = xotr.tile([P, OUTD], f32, tag="tneg")
        nc.vector.tensor_scalar_min(tneg[:], outn[:], 0.0)
        texp = xotr.tile([P, OUTD], f32, tag="texp")
        nc.scalar.activation(texp[:], tneg[:], Act.Exp)
        xo_m = xop.tile([P, OUTD], bf16, tag="xo")
        nc.vector.scalar_tensor_tensor(
            out=xo_m[:], in0=texp[:], scalar=-1.0, in1=outn[:],
            op0=Alu.add, op1=Alu.max,
        )

        # ---- y = elu(dot(xo[0], xo_m[j])) ----
        xo0p = tpp.tile([P, IND], f32, tag="tp")
        nc.tensor.matmul(
            xo0p[:, :OUTD], lhsT=ones_b[:], rhs=xo_b[0:1, :], start=True, stop=True,
            skip_group_check=True,
        )
        xo0s = const.tile([P, OUTD], bf16, tag="xo0s")
        nc.vector.tensor_copy(xo0s[:], xo0p[:, :OUTD])
        dscr = scrp.tile([P, OUTD], f32, tag="dscr")
        d_sb = smallp.tile([P, 1], f32, tag="d_sb")
        nc.vector.scalar_tensor_tensor(
            out=dscr[:], in0=xo_m[:], scalar=0.0, in1=xo0s[:],
            op0=Alu.bypass, op1=Alu.mult, accum_out=d_sb[:],
        )
        yneg = smallp.tile([P, 1], f32, tag="yneg")
        nc.vector.tensor_scalar_min(yneg[:], d_sb[:], 0.0)
        yexp = smallp.tile([P, 1], f32, tag="yexp")
        nc.scalar.activation(yexp[:], yneg[:], Act.Exp)
        y_bf = smallp.tile([P, 1], bf16, tag="y_bf")
        nc.vector.scalar_tensor_tensor(
            out=y_bf[:], in0=yexp[:], scalar=-1.0, in1=d_sb[:],
            op0=Alu.add, op1=Alu.max,
        )
        # write y as a contiguous [1, P] row (column DMA has a huge
        # HBM completion delay that the kernel-tail barrier waits out)
        yrp = tpp.tile([P, P], bf16, tag="tp")
        nc.tensor.transpose(yrp[:1, :], y_bf[:], ident[:])
        y_row = smallp.tile([1, P], f32, tag="y_row")
        nc.vector.tensor_copy(y_row[:], yrp[:1, :P])
        nc.sync.dma_start(y_out[:], y_row[:])

    nc.compile()
    return nc


def _get_program(n_nodes, T_b, T_d):
    key = (n_nodes, T_b, T_d)
    if key not in _CACHE:
        _CACHE[key] = _build_program(n_nodes, T_b, T_d)
    return _CACHE[key]


def _pack_cols(vals, T, pad, dtype):
    """[n] -> [P, T] column-per-tile layout (tile t, lane p) = vals[t*P+p]."""
    npad = T * P - len(vals)
    v = np.concatenate([vals, np.full(npad, pad, vals.dtype)])
    return np.ascontiguousarray(v.reshape(T, P).T).astype(dtype)


def _onehot_blocks(dst_cols):
    """dst_cols [P, T] -> (S [P, T*P], ST [P, T*P]) one-hot bf16 blocks.
    S_t[e, j] = (dst[e, t] == j); ST_t = S_t^T. dst==P rows are all-zero."""
    Pn = P
    T = dst_cols.shape[1]
    S = np.zeros((Pn, T * Pn), dtype=BF16)
    ST = np.zeros((Pn, T * Pn), dtype=BF16)
    e_idx, t_idx = np.nonzero(dst_cols < Pn)
    j_idx = dst_cols[e_idx, t_idx]
    S[e_idx, t_idx * Pn + j_idx] = 1
    ST[j_idx, t_idx * Pn + e_idx] = 1
    return np.ascontiguousarray(S), np.ascontiguousarray(ST)


def prepare(x, edge_index, W, att_src, att_dst, bias, item_len):
    """Python-side edge partitioning; returns (nc, in_maps, item_len)."""
    item_len = int(np.asarray(item_len))
    x = np.ascontiguousarray(np.asarray(x, np.float32))
    W = np.ascontiguousarray(np.asarray(W, np.float32))
    att_src = np.asarray(att_src, np.float32)
    att_dst = np.asarray(att_dst, np.float32)
    bias = np.asarray(bias, np.float32)
    n_nodes = x.shape[0]
    assert x.shape[1] == IND and W.shape == (IND, OUTD)
    assert item_len <= N_CORES * P, "kernel supports item_len <= 1024"

    src = np.asarray(edge_index[0])
    dst = np.asarray(edge_index[1])
    keep = dst < item_len
    src_f = src[keep].astype(np.int32)
    dst_f = dst[keep].astype(np.int32)

    # dst==0 block (graph edges + node-0 self loop), shared by all cores
    sel0 = dst_f == 0
    b_src = np.concatenate([src_f[sel0], np.zeros(1, np.int32)])
    T_b = max(1, math.ceil(len(b_src) / P))

    blk = dst_f // P
    order = np.argsort(blk, kind="stable")
    src_f = src_f[order]
    dst_f = dst_f[order]
    blk = blk[order]
    bounds = np.searchsorted(blk, np.arange(N_CORES + 1))
    T_d = max(
        1, max(math.ceil(int(bounds[k + 1] - bounds[k]) / P) for k in range(N_CORES))
    )

    nc = _get_program(n_nodes, T_b, T_d)

    # host weight preprocessing
    ws_r = np.ascontiguousarray((W @ att_src).astype(np.float32).reshape(1, IND))
    wd_r = np.ascontiguousarray((W @ att_dst).astype(np.float32).reshape(1, IND))
    w_bf = np.ascontiguousarray(W.astype(BF16))
    ident = np.eye(P, dtype=np.float32).astype(BF16)
    x0 = np.ascontiguousarray(x[0:1])
    bias_r = np.ascontiguousarray(bias.reshape(1, OUTD))

    b_eidx = _pack_cols(b_src, T_b, 0, np.int32)
    b_dst = _pack_cols(np.zeros(len(b_src), np.int32), T_b, P, np.int32)
    T_idx = T_b + T_d

    in_maps = []
    for k in range(N_CORES):
        lo, hi = bounds[k], bounds[k + 1]
        es = src_f[lo:hi]
        ed = dst_f[lo:hi] - k * P
        dst_cols = [b_dst, _pack_cols(ed, T_d, P, np.int32)]
        eidx = _pack_cols(es, T_d, 0, np.int32)
        self_dst = np.arange(P, dtype=np.int32)
        if (k + 1) * P > item_len:
            self_dst = np.where(
                np.arange(k * P, (k + 1) * P) < item_len, self_dst, P
            ).astype(np.int32)
        dst_cols.append(self_dst[:, None])
        dst_all = np.concatenate(dst_cols, axis=1)
        S, ST = _onehot_blocks(dst_all)
        xself = np.ascontiguousarray(
            x[np.minimum(np.arange(k * P, (k + 1) * P), n_nodes - 1)]
        )
        m = {
            "x_in": x,
            "idx_in": np.ascontiguousarray(np.concatenate(
                [b_eidx, eidx,
                 np.zeros((P, T_idx - T_b - T_d), np.int32)], axis=1)),
            "xself_in": xself,
            "x0_in": x0,
            "s_in": S,
            "st_in": ST,
            "ws_in": ws_r,
            "wd_in": wd_r,
            "w_in": w_bf,
            "bias_in": bias_r,
            "ident_in": ident,
        }
        in_maps.append(m)
    return nc, in_maps, item_len


def assemble(results, item_len):
    y_all = np.concatenate(
        [np.asarray(results[k]["y_out"], np.float32).ravel() for k in range(N_CORES)]
    )
    return y_all[1:item_len].astype(np.float32)


def kernel(x, edge_index, W, att_src, att_dst, bias, item_len):
    from concourse import bass_utils

    nc, in_maps, item_len = prepare(
        x, edge_index, W, att_src, att_dst, bias, item_len
    )
    res = bass_utils.run_bass_kernel_spmd(nc, in_maps, core_ids=list(range(N_CORES)))
    return assemble(res.results, item_len)


# revision 12
# speedup vs baseline: 1.1683x; 1.1316x over previous
"""GAT message-passing kernel for Trainium2 (8 NeuronCores, Bass/Tile).

Strategy (edge-parallel, dst-block partitioning): the model output
y = elu(sum(xo[0] * xo[1:item_len], 1)) depends only on output rows
0..item_len-1, so only edges with dst < item_len contribute (~33K of
3.2M edges). Core k owns dst rows [128k, 128k+128); every core also
processes the dst==0 edges so xo[0] is available locally.

Gather: Q7 SWDGE descriptor generation runs at ~8.5 ns/row regardless
of instruction (indirect DMA or the dma_gather ucode), so the x[src]
gather stream (~4.3K rows/core) is the kernel's hard critical path
(~48 us). GATHER_MODE picks the mechanism: "ind" (default) issues one
classic indirect DMA per 128-edge tile (i32 indices; multi-index
batching is corrupted by the HW ucode, and dma_gather pays an ~11 us
one-time IRAM load plus DVE-contention inflation, so per-tile indirect
measured fastest). The self-loop tile is a direct DMA of the core's own
row block. All per-edge compute is sized to hide under the gather
stream, balanced across engines (DVE also throttles the Q7 descriptor
stream via the shared SBUF port, so DVE bytes are minimized):

  tensor:  ad_e column = S_t^T @ a_d        (host-shipped one-hot S)
  scalar/
  vector:  xbf = bf16(x_src)                (cast, engines alternated)
  vector:  a_s = rowsum(xbf * w_s)          (bf16 stt with accumulate)
  batched: p = exp(leaky(a_s + a_d))        (per tile-group)
  scalar:  rhs = [bf16(p * x_src) | p]      (cast fused with p scale)
  tensor:  acc += S_t^T @ rhs               (bf16 matmul, f32 PSUM)

The group pipeline is software-skewed (stage A of group g+1 enqueues
before stage B of group g) with tapered final groups, and the main
accumulator is split in two so half the (u@W)/z epilogue runs
mid-stream. out = (u@W)/z + bias; xo = elu(out); y = elu(xo_m @ xo[0]).
The host precomputes W@att_src / W@att_dst, casts W to bf16, and builds
the one-hot S / S^T blocks (index metadata only; x is never touched
beyond contiguous slicing).
"""
import math

import numpy as np
import ml_dtypes

P = 128
N_CORES = 8
NEG_SLOPE = 0.2
IND = 256
OUTD = 128
W258 = IND + 2  # rhs tile stride: 256 data + 1 p column + 1 pad
BF16 = ml_dtypes.bfloat16
BUCKET = 32768  # fixed dma_gather rebase step (int16 index range)
MAXCHUNK = 8    # dma_gather HW limit: 1024 indices per call
KGRP = 4        # tiles per compute group (small-op batching)
GATHER_MODE = "ind"  # "ind": per-tile indirect DMA; "dg": bucketed dma_gather

_CACHE = {}


def _chunk_plan(bucket_tiles):
    """[(base_bucket, ntiles)] per dma_gather call, capped at MAXCHUNK."""
    plan = []
    for b, nt in enumerate(bucket_tiles):
        rest = nt
        while rest > MAXCHUNK:
            plan.append((b, MAXCHUNK))
            rest -= MAXCHUNK
        if rest:
            plan.append((b, rest))
    return tuple(plan)


def _build_program(n_nodes, T_b, chunks, mode):
    import concourse.bass as bass
    import concourse.bacc as bacc
    import concourse.tile as tile
    import concourse.mybir as mybir
    from contextlib import ExitStack

    f32 = mybir.dt.float32
    bf16 = mybir.dt.bfloat16
    i32 = mybir.dt.int32
    i16 = mybir.dt.int16
    Alu = mybir.AluOpType
    Act = mybir.ActivationFunctionType

    T_d = sum(nt for _, nt in chunks)
    T_all = T_b + T_d + 1  # b tiles + data tiles + self tile
    ncmax = max(nt for _, nt in chunks)

    nc = bacc.Bacc(
        "TRN2", target_bir_lowering=False, debug=False, num_devices=N_CORES
    )
    x_in = nc.dram_tensor("x_in", [n_nodes, IND], f32, kind="ExternalInput").ap()
    xself_in = nc.dram_tensor("xself_in", [P, IND], f32, kind="ExternalInput").ap()
    x0_in = nc.dram_tensor("x0_in", [1, IND], f32, kind="ExternalInput").ap()
    s_in = nc.dram_tensor("s_in", [P, T_all * P], bf16, kind="ExternalInput").ap()
    st_in = nc.dram_tensor("st_in", [P, T_all * P], bf16, kind="ExternalInput").ap()
    ws_in = nc.dram_tensor("ws_in", [1, IND], f32, kind="ExternalInput").ap()
    wd_in = nc.dram_tensor("wd_in", [1, IND], f32, kind="ExternalInput").ap()
    w_in = nc.dram_tensor("w_in", [IND, OUTD], bf16, kind="ExternalInput").ap()
    bias_in = nc.dram_tensor("bias_in", [1, OUTD], f32, kind="ExternalInput").ap()
    ident_in = nc.dram_tensor("ident_in", [P, P], bf16, kind="ExternalInput").ap()
    beidx_in = nc.dram_tensor("beidx_in", [P, T_b], i32, kind="ExternalInput").ap()
    if mode == "dg":
        gidx_ins = [
            nc.dram_tensor(f"gidx{g}_in", [P, nt * 8], i16, kind="ExternalInput").ap()
            for g, (_, nt) in enumerate(chunks)
        ]
    else:
        eidx_in = nc.dram_tensor("eidx_in", [P, T_d], i32, kind="ExternalInput").ap()
    y_out = nc.dram_tensor("y_out", [1, P], f32, kind="ExternalOutput").ap()

    with tile.TileContext(nc) as tc, ExitStack() as ctx:
        const = ctx.enter_context(tc.tile_pool(name="const", bufs=1))
        idxp = ctx.enter_context(tc.tile_pool(name="idx", bufs=1))
        nxg = len(chunks) if mode == "dg" else T_d
        xgp = ctx.enter_context(tc.tile_pool(name="xg", bufs=nxg))
        xgbp = ctx.enter_context(tc.tile_pool(name="xgb", bufs=max(T_b, 1)))
        xbfp = ctx.enter_context(tc.tile_pool(name="xbf", bufs=KGRP + 3))
        rhsp = ctx.enter_context(tc.tile_pool(name="rhs", bufs=KGRP + 3))
        scrp = ctx.enter_context(tc.tile_pool(name="scr", bufs=3))
        grpp = ctx.enter_context(tc.tile_pool(name="grp", bufs=12))
        smallp = ctx.enter_context(tc.tile_pool(name="small", bufs=8))
        utp = ctx.enter_context(tc.tile_pool(name="ut", bufs=2))
        xotr = ctx.enter_context(tc.tile_pool(name="xotr", bufs=4))
        xop = ctx.enter_context(tc.tile_pool(name="xo", bufs=2))
        accsb = ctx.enter_context(tc.tile_pool(name="accsb", bufs=2))
        # PSUM banks: acc 2 + tp 2 + outp 2 + adp 2 = 8
        accp = ctx.enter_context(tc.tile_pool(name="acc", bufs=2, space="PSUM"))
        tpp = ctx.enter_context(tc.tile_pool(name="tp", bufs=2, space="PSUM"))
        outpp = ctx.enter_context(tc.tile_pool(name="outp", bufs=2, space="PSUM"))
        adpp = ctx.enter_context(tc.tile_pool(name="adp", bufs=2, space="PSUM"))

        # ---- index DMAs first: the gather stream depends only on these ----
        beidx_t = idxp.tile([P, T_b], i32, tag="beidx")
        nc.scalar.dma_start(beidx_t[:], beidx_in[:])
        if mode == "dg":
            gidx_ts = []
            for g, (_, nt) in enumerate(chunks):
                t = idxp.tile([P, nt * 8], i16, tag=f"gidx{g}")
                nc.scalar.dma_start(t[:], gidx_ins[g][:])
                gidx_ts.append(t)
            # dummy gather first: absorbs the one-time Q7 ucode IRAM load
            dummy_idx = idxp.tile([P, 8], i16, tag="didx")
            nc.vector.memset(dummy_idx[:], 0)
            dummy_out = idxp.tile([P, IND], f32, tag="dout")
            nc.gpsimd.dma_gather(
                out_ap=dummy_out[:].rearrange("p (t c) -> p t c", c=IND),
                in_ap=x_in[0:BUCKET, :],
                idxs_ap=dummy_idx[:],
                num_idxs=P,
                num_idxs_reg=P,
                elem_size=IND,
            )
        else:
            eidx_t = idxp.tile([P, T_d], i32, tag="eidx")
            nc.scalar.dma_start(eidx_t[:], eidx_in[:])

        # dst==0 block: classic per-tile indirect DMA (full-range i32 idx)
        xg_b = []
        for tb in range(T_b):
            xb = xgbp.tile([P, IND], f32, tag="xgb")
            nc.gpsimd.indirect_dma_start(
                out=xb[:],
                out_offset=None,
                in_=x_in[:],
                in_offset=bass.IndirectOffsetOnAxis(
                    ap=beidx_t[:, tb : tb + 1], axis=0
                ),
            )
            xg_b.append(xb)
        # data tiles
        xg_chunks = []
        if mode == "dg":
            for g, (b, nt) in enumerate(chunks):
                base = b * BUCKET
                hi = min(base + BUCKET, n_nodes)
                xg = xgp.tile([P, ncmax * IND], f32, tag="xg")
                nc.gpsimd.dma_gather(
                    out_ap=xg[:, 0 : nt * IND].rearrange("p (t c) -> p t c", c=IND),
                    in_ap=x_in[base:hi, :],
                    idxs_ap=gidx_ts[g][:],
                    num_idxs=nt * P,
                    num_idxs_reg=nt * P,
                    elem_size=IND,
                )
                xg_chunks.append((xg, nt))
        else:
            for t in range(T_d):
                xg = xgp.tile([P, IND], f32, tag="xg")
                nc.gpsimd.indirect_dma_start(
                    out=xg[:],
                    out_offset=None,
                    in_=x_in[:],
                    in_offset=bass.IndirectOffsetOnAxis(
                        ap=eidx_t[:, t : t + 1], axis=0
                    ),
                )
                xg_chunks.append((xg, 1))

        # ---- remaining input DMAs (small ones first; S/S^T are ~1MB) ----
        xself_t = const.tile([P, IND], f32, tag="xself")
        nc.sync.dma_start(xself_t[:], xself_in[:])
        x0_t = const.tile([1, IND], f32, tag="x0")
        nc.sync.dma_start(x0_t[:], x0_in[:])
        ws_t = const.tile([1, IND], f32, tag="ws_t")
        nc.sync.dma_start(ws_t[:], ws_in[:])
        wd_t = const.tile([1, IND], f32, tag="wd_t")
        nc.sync.dma_start(wd_t[:], wd_in[:])
        W0 = const.tile([P, OUTD], bf16, tag="W0")
        nc.sync.dma_start(W0[:], w_in[0:P, :])
        W1 = const.tile([P, OUTD], bf16, tag="W1")
        nc.sync.dma_start(W1[:], w_in[P : 2 * P, :])
        bias_t = const.tile([1, OUTD], f32, tag="bias")
        nc.sync.dma_start(bias_t[:], bias_in[:])
        ident = const.tile([P, P], bf16, tag="ident")
        nc.sync.dma_start(ident[:], ident_in[:])
        s_t = const.tile([P, T_all * P], bf16, tag="s_t")
        nc.sync.dma_start(s_t[:], s_in[:])
        st_t = const.tile([P, T_all * P], bf16, tag="st_t")
        nc.sync.dma_start(st_t[:], st_in[:])

        ones_f = const.tile([1, P], f32, tag="ones_f")
        nc.vector.memset(ones_f[:], 1.0)
        ones_b = const.tile([1, P], bf16, tag="ones_b")
        nc.vector.memset(ones_b[:], 1.0)

        # ---- prologue: broadcast weights / attention rows (f32) ----
        def bcast_f32(row_ap, width):
            bp = tpp.tile([P, IND], f32, tag="tp")
            nc.tensor.matmul(
                bp[:, :width], lhsT=ones_f[:], rhs=row_ap, start=True, stop=True,
                skip_group_check=True,
            )
            return bp

        wsp = bcast_f32(ws_t[:], IND)
        wsb = const.tile([P, IND], bf16, tag="wsb")
        nc.vector.tensor_copy(wsb[:], wsp[:, :IND])
        wdp = bcast_f32(wd_t[:], IND)
        wdb = const.tile([P, IND], f32, tag="wdb")
        nc.vector.tensor_copy(wdb[:], wdp[:, :IND])
        bp = bcast_f32(bias_t[:], OUTD)
        bias_b = const.tile([P, OUTD], f32, tag="bias_b")
        nc.vector.tensor_copy(bias_b[:], bp[:, :OUTD])

        # a_d per local row (bf16 column feeds the per-tile S^T matmuls)
        scr0 = scrp.tile([P, IND], f32, tag="scrf")
        ad_col = smallp.tile([P, 1], f32, tag="ad_col")
        nc.vector.scalar_tensor_tensor(
            out=scr0[:], in0=xself_t[:], scalar=0.0, in1=wdb[:],
            op0=Alu.bypass, op1=Alu.mult, accum_out=ad_col[:],
        )
        ad_bf = const.tile([P, 1], bf16, tag="ad_bf")
        nc.vector.tensor_copy(ad_bf[:], ad_col[:])

        # a_d[0] (node 0), broadcast to all partitions (for the dst==0 block)
        scr1 = scrp.tile([P, IND], f32, tag="scrf")
        ad0_f = smallp.tile([1, 1], f32, tag="ad0_f")
        nc.vector.scalar_tensor_tensor(
            out=scr1[0:1, :], in0=x0_t[:], scalar=0.0, in1=wdb[0:1, :],
            op0=Alu.bypass, op1=Alu.mult, accum_out=ad0_f[:],
        )
        ad0_bf = smallp.tile([1, 1], bf16, tag="ad0_bf")
        nc.vector.tensor_copy(ad0_bf[:], ad0_f[:])
        ad0p = tpp.tile([P, IND], f32, tag="tp")
        nc.tensor.matmul(
            ad0p[:, 0:1], lhsT=ones_b[:], rhs=ad0_bf[:], start=True, stop=True,
            skip_group_check=True,
        )
        ad0_col = smallp.tile([P, 1], f32, tag="ad0_col")
        nc.vector.tensor_copy(ad0_col[:], ad0p[:, 0:1])

        acc_b = accp.tile([P, IND + 1], f32, tag="acc")
        acc_m1 = accp.tile([P, IND + 1], f32, tag="acc")
        acc_m2 = accp.tile([P, IND + 1], f32, tag="acc")

        # ---- epilogue helpers ----
        def out_phase(acc, outp, first, tag):
            u_bf = accsb.tile([P, IND], bf16, tag="u_bf")
            nc.vector.tensor_copy(u_bf[:], acc[:, 0:IND])
            z = smallp.tile([P, 1], f32, tag=f"z{tag}")
            nc.vector.tensor_scalar_add(z[:], acc[:, IND : IND + 1], 1e-30)
            for ci in range(2):
                tp = tpp.tile([P, P], bf16, tag="tp")
                nc.tensor.transpose(tp[:], u_bf[:, ci * P : (ci + 1) * P], ident[:])
                uT = utp.tile([P, P], bf16, tag="uT")
                nc.vector.tensor_copy(uT[:], tp[:])
                nc.tensor.matmul(
                    outp[:], lhsT=uT[:], rhs=(W0 if ci == 0 else W1)[:],
                    start=(first and ci == 0), stop=((not first) and ci == 1),
                    skip_group_check=True,
                )
            return z

        # ---- epilogue helper: xo = elu((u@W)/z + bias) ----
        def out_block(acc, tag):
            u_bf = accsb.tile([P, IND], bf16, tag="u_bf")
            nc.vector.tensor_copy(u_bf[:], acc[:, 0:IND])
            z = smallp.tile([P, 1], f32, tag=f"z{tag}")
            nc.vector.tensor_scalar_add(z[:], acc[:, IND : IND + 1], 1e-30)
            outp = outpp.tile([P, OUTD], f32, tag="outp")
            for ci in range(2):
                tp = tpp.tile([P, P], bf16, tag="tp")
                nc.tensor.transpose(tp[:], u_bf[:, ci * P : (ci + 1) * P], ident[:])
                uT = utp.tile([P, P], bf16, tag="uT")
                nc.vector.tensor_copy(uT[:], tp[:])
                nc.tensor.matmul(
                    outp[:], lhsT=uT[:], rhs=(W0 if ci == 0 else W1)[:],
                    start=(ci == 0), stop=(ci == 1), skip_group_check=True,
                )
            rz = smallp.tile([P, 1], f32, tag=f"rz{tag}")
            nc.vector.reciprocal(rz[:], z[:])
            outn = xotr.tile([P, OUTD], f32, tag="outn")
            nc.vector.scalar_tensor_tensor(
                out=outn[:], in0=outp[:], scalar=rz[:], in1=bias_b[:],
                op0=Alu.mult, op1=Alu.add,
            )
            tneg = xotr.tile([P, OUTD], f32, tag="tneg")
            nc.vector.tensor_scalar_min(tneg[:], outn[:], 0.0)
            texp = xotr.tile([P, OUTD], f32, tag="texp")
            nc.scalar.activation(texp[:], tneg[:], Act.Exp)
            xo = xop.tile([P, OUTD], bf16, tag="xo")
            nc.vector.scalar_tensor_tensor(
                out=xo[:], in0=texp[:], scalar=-1.0, in1=outn[:],
                op0=Alu.add, op1=Alu.max,
            )
            return xo

        # slot -> f32 source AP of the gathered/self rows
        def slot_src(s):
            if s < T_b:
                return xg_b[s][:]
            if s == T_b + T_d:
                return xself_t[:]
            d = s - T_b
            for xg, nt in xg_chunks:
                if d < nt:
                    return xg[:, d * IND : (d + 1) * IND]
                d -= nt
            raise AssertionError

        # ---- main stream: compute groups over all slots, software-
        # pipelined with a one-group skew (stage A of group g+1 enqueues
        # before stage B of group g) so the in-order engine queues never
        # head-of-line block on a not-yet-gathered tile ----
        groups = [list(range(T_b))]
        rest = list(range(T_b, T_all))
        taper = [2, 1] if len(rest) > KGRP + 3 else []
        head = len(rest) - sum(taper)
        for i in range(0, head, KGRP):
            groups.append(rest[i : min(i + KGRP, head)])
        pos = head
        for tsz in taper:
            groups.append(rest[pos : pos + tsz])
            pos += tsz
        ngroups = len(groups)
        n_endgame = sum(taper) + KGRP  # slots in the drain window

        state = {}

        def stage_a(gi):
            slots = groups[gi]
            as_g = grpp.tile([P, KGRP], f32, tag="as")
            adp = adpp.tile([P, KGRP], f32, tag="adp")
            xbfs = []
            for j, s in enumerate(slots):
                src = slot_src(s)
                if s >= T_b:
                    nc.tensor.matmul(
                        adp[:, j : j + 1], lhsT=st_t[:, s * P : (s + 1) * P],
                        rhs=ad_bf[:], start=True, stop=True, skip_group_check=True,
                    )
                # plain bf16 cast of the gathered rows; alternate engines so
                # neither DVE (which throttles the Q7 descriptor stream via
                # the shared SBUF port) nor ACT becomes the wall
                xbf = xbfp.tile([P, IND], bf16, tag="xbf")
                if s % 2 == 0 and s < T_all - n_endgame:
                    nc.scalar.activation(xbf[:], src, Act.Copy)
                else:
                    nc.vector.tensor_copy(xbf[:], src)
                scr = scrp.tile([P, IND], bf16, tag="scr")
                nc.vector.scalar_tensor_tensor(
                    out=scr[:], in0=xbf[:], scalar=0.0, in1=wsb[:],
                    op0=Alu.bypass, op1=Alu.mult,
                    accum_out=as_g[:, j : j + 1],
                )
                xbfs.append(xbf)
            state[gi] = (as_g, adp, xbfs)

        def stage_b(gi):
            slots = groups[gi]
            k = len(slots)
            as_g, adp, xbfs = state.pop(gi)
            ad_g = grpp.tile([P, KGRP], f32, tag="ad")
            if gi == 0:
                for j in range(k):
                    nc.vector.tensor_copy(ad_g[:, j : j + 1], ad0_col[:])
            else:
                nc.vector.tensor_copy(ad_g[:, 0:k], adp[:, 0:k])
            v_g = grpp.tile([P, KGRP], f32, tag="v")
            nc.vector.tensor_tensor(
                out=v_g[:, 0:k], in0=as_g[:, 0:k], in1=ad_g[:, 0:k], op=Alu.add
            )
            e_g = grpp.tile([P, KGRP], f32, tag="e")
            nc.vector.scalar_tensor_tensor(
                out=e_g[:, 0:k], in0=v_g[:, 0:k], scalar=NEG_SLOPE,
                in1=v_g[:, 0:k], op0=Alu.mult, op1=Alu.max,
            )
            p_g = grpp.tile([P, KGRP], f32, tag="p")
            nc.scalar.activation(p_g[:, 0:k], e_g[:, 0:k], Act.Exp)
            for j, s in enumerate(slots):
                rhs_bf = rhsp.tile([P, W258], bf16, tag="rhs")
                pcol = p_g[:, j : j + 1]
                nc.vector.tensor_copy(rhs_bf[:, IND : IND + 1], pcol)
                nc.scalar.activation(
                    rhs_bf[:, 0:IND], xbfs[j][:], Act.Copy, scale=pcol
                )
                rhs = rhs_bf[:, 0 : IND + 1]
                if s < T_b:
                    nc.tensor.matmul(
                        acc_b[:], lhsT=s_t[:, s * P : (s + 1) * P], rhs=rhs,
                        start=(s == 0), stop=(s == T_b - 1), skip_group_check=True,
                    )
                else:
                    accx = acc_m1 if s < mid_slot else acc_m2
                    nc.tensor.matmul(
                        accx[:], lhsT=s_t[:, s * P : (s + 1) * P], rhs=rhs,
                        start=(s in (T_b, mid_slot)),
                        stop=(s in (mid_slot - 1, T_all - 1)),
                        skip_group_check=True,
                    )
            if gi == 0:
                # dst==0 block complete: fold its epilogue under the stream
                state["xo_b"] = out_block(acc_b, "b")

        mid_gi = 1 + (ngroups - 1) // 2
        mid_slot = groups[mid_gi][0]
        outp_m = outpp.tile([P, OUTD], f32, tag="outp")
        z1 = None
        for gi in range(ngroups):
            stage_a(gi)
            stage_b(gi)
            if gi == mid_gi:
                z1 = out_phase(acc_m1, outp_m, True, "m1")
        if False:
            stage_b(ngroups - 1)
        xo_b = state["xo_b"]

        z2 = out_phase(acc_m2, outp_m, False, "m2")
        zz = smallp.tile([P, 1], f32, tag="zz")
        nc.vector.tensor_tensor(out=zz[:], in0=z1[:], in1=z2[:], op=Alu.add)
        rz = smallp.tile([P, 1], f32, tag="rzm")
        nc.vector.reciprocal(rz[:], zz[:])
        outn = xotr.tile([P, OUTD], f32, tag="outn")
        nc.vector.scalar_tensor_tensor(
            out=outn[:], in0=outp_m[:], scalar=rz[:], in1=bias_b[:],
            op0=Alu.mult, op1=Alu.add,
        )
        tneg = xotr.tile([P, OUTD], f32, tag="tneg")
        nc.vector.tensor_scalar_min(tneg[:], outn[:], 0.0)
        texp = xotr.tile([P, OUTD], f32, tag="texp")
        nc.scalar.activation(texp[:], tneg[:], Act.Exp)
        xo_m = xop.tile([P, OUTD], bf16, tag="xo")
        nc.vector.scalar_tensor_tensor(
            out=xo_m[:], in0=texp[:], scalar=-1.0, in1=outn[:],
            op0=Alu.add, op1=Alu.max,
        )

        # ---- y = elu(dot(xo[0], xo_m[j])) ----
        xo0p = tpp.tile([P, IND], f32, tag="tp")
        nc.tensor.matmul(
            xo0p[:, :OUTD], lhsT=ones_b[:], rhs=xo_b[0:1, :], start=True, stop=True,
            skip_group_check=True,
        )
        xo0s = const.tile([P, OUTD], bf16, tag="xo0s")
        nc.vector.tensor_copy(xo0s[:], xo0p[:, :OUTD])
        dscr = scrp.tile([P, OUTD], f32, tag="dscr")
        d_sb = smallp.tile([P, 1], f32, tag="d_sb")
        nc.vector.scalar_tensor_tensor(
            out=dscr[:], in0=xo_m[:], scalar=0.0, in1=xo0s[:],
            op0=Alu.bypass, op1=Alu.mult, accum_out=d_sb[:],
        )
        yneg = smallp.tile([P, 1], f32, tag="yneg")
        nc.vector.tensor_scalar_min(yneg[:], d_sb[:], 0.0)
        yexp = smallp.tile([P, 1], f32, tag="yexp")
        nc.scalar.activation(yexp[:], yneg[:], Act.Exp)
        y_bf = smallp.tile([P, 1], bf16, tag="y_bf")
        nc.vector.scalar_tensor_tensor(
            out=y_bf[:], in0=yexp[:], scalar=-1.0, in1=d_sb[:],
            op0=Alu.add, op1=Alu.max,
        )
        # write y as a contiguous [1, P] row (column DMA has a huge
        # HBM completion delay that the kernel-tail barrier waits out)
        yrp = tpp.tile([P, P], bf16, tag="tp")
        nc.tensor.transpose(yrp[:1, :], y_bf[:], ident[:])
        y_row = smallp.tile([1, P], f32, tag="y_row")
        nc.vector.tensor_copy(y_row[:], yrp[:1, :P])
        nc.sync.dma_start(y_out[:], y_row[:])

    nc.compile()
    return nc


def _get_program(n_nodes, T_b, chunks, mode):
    key = (n_nodes, T_b, chunks, mode)
    if key not in _CACHE:
        _CACHE[key] = _build_program(n_nodes, T_b, chunks, mode)
    return _CACHE[key]


def _pack_cols(vals, T, pad, dtype):
    """[n] -> [P, T] column-per-tile layout (tile t, lane p) = vals[t*P+p]."""
    npad = T * P - len(vals)
    v = np.concatenate([vals, np.full(npad, pad, vals.dtype)])
    return np.ascontiguousarray(v.reshape(T, P).T).astype(dtype)


def _wrap16(idx, nidx):
    """int16 dma_gather index layout: value i at [i%16, i//16], the
    16-partition block replicated to 128 partitions."""
    npad = nidx - len(idx)
    v = np.concatenate([idx, np.zeros(npad, np.int16)])
    a = v.reshape(nidx // 16, 16).T
    return np.ascontiguousarray(np.tile(a, (8, 1)))


def _onehot_blocks(dst_cols):
    """dst_cols [P, T] -> (S [P, T*P], ST [P, T*P]) one-hot bf16 blocks.
    S_t[e, j] = (dst[e, t] == j); ST_t = S_t^T. dst==P rows are all-zero."""
    Pn = P
    T = dst_cols.shape[1]
    S = np.zeros((Pn, T * Pn), dtype=BF16)
    ST = np.zeros((Pn, T * Pn), dtype=BF16)
    e_idx, t_idx = np.nonzero(dst_cols < Pn)
    j_idx = dst_cols[e_idx, t_idx]
    S[e_idx, t_idx * Pn + j_idx] = 1
    ST[j_idx, t_idx * Pn + e_idx] = 1
    return np.ascontiguousarray(S), np.ascontiguousarray(ST)


def prepare(x, edge_index, W, att_src, att_dst, bias, item_len):
    """Python-side edge partitioning; returns (nc, in_maps, item_len)."""
    item_len = int(np.asarray(item_len))
    x = np.ascontiguousarray(np.asarray(x, np.float32))
    W = np.ascontiguousarray(np.asarray(W, np.float32))
    att_src = np.asarray(att_src, np.float32)
    att_dst = np.asarray(att_dst, np.float32)
    bias = np.asarray(bias, np.float32)
    n_nodes = x.shape[0]
    assert x.shape[1] == IND and W.shape == (IND, OUTD)
    assert item_len <= N_CORES * P, "kernel supports item_len <= 1024"

    src = np.asarray(edge_index[0])
    dst = np.asarray(edge_index[1])
    keep = dst < item_len
    src_f = src[keep].astype(np.int32)
    dst_f = dst[keep].astype(np.int32)

    # dst==0 block (graph edges + node-0 self loop), shared by all cores
    sel0 = dst_f == 0
    b_src = np.concatenate([src_f[sel0], np.zeros(1, np.int32)])
    T_b = max(1, math.ceil(len(b_src) / P))

    blk = dst_f // P
    order = np.argsort(blk, kind="stable")
    src_f = src_f[order]
    dst_f = dst_f[order]
    blk = blk[order]
    bounds = np.searchsorted(blk, np.arange(N_CORES + 1))

    mode = GATHER_MODE
    n_buckets = math.ceil(n_nodes / BUCKET)
    cores = []
    tile_counts = []
    for k in range(N_CORES):
        lo, hi = bounds[k], bounds[k + 1]
        es = src_f[lo:hi]
        ed = dst_f[lo:hi] - k * P
        if mode == "dg":
            o = np.argsort(es, kind="stable")
            es, ed = es[o], ed[o]
            bkt = es // BUCKET
            bb = np.searchsorted(bkt, np.arange(n_buckets + 1))
            cnt = np.diff(bb)
            tiles = [math.ceil(c / P) for c in cnt]
            tile_counts.append(tiles)
        else:
            bb = None
            tile_counts.append([math.ceil(len(es) / P)])
        cores.append((es, ed, bb))
    if mode == "dg":
        bucket_tiles = [max(tc[b] for tc in tile_counts) for b in range(n_buckets)]
        chunks = _chunk_plan(bucket_tiles)
    else:
        bucket_tiles = None
        chunks = ((0, max(1, max(tc[0] for tc in tile_counts))),)

    nc = _get_program(n_nodes, T_b, chunks, mode)

    # host weight preprocessing
    ws_r = np.ascontiguousarray((W @ att_src).astype(np.float32).reshape(1, IND))
    wd_r = np.ascontiguousarray((W @ att_dst).astype(np.float32).reshape(1, IND))
    w_bf = np.ascontiguousarray(W.astype(BF16))
    ident = np.eye(P, dtype=np.float32).astype(BF16)
    x0 = np.ascontiguousarray(x[0:1])
    bias_r = np.ascontiguousarray(bias.reshape(1, OUTD))

    b_eidx = _pack_cols(b_src, T_b, 0, np.int32)
    b_dst = _pack_cols(np.zeros(len(b_src), np.int32), T_b, P, np.int32)

    in_maps = []
    for k in range(N_CORES):
        es, ed, bb = cores[k]
        dst_cols = [b_dst]
        gidx = {}
        if mode == "dg":
            for b in range(n_buckets):
                nt_b = bucket_tiles[b]
                if nt_b == 0:
                    continue
                e_b = es[bb[b] : bb[b + 1]] - b * BUCKET
                d_b = ed[bb[b] : bb[b + 1]]
                dst_cols.append(_pack_cols(d_b, nt_b, P, np.int32))
                idx16 = e_b.astype(np.int16)
                t0 = 0
                for g, (cb, nt) in enumerate(chunks):
                    if cb != b:
                        continue
                    seg = idx16[t0 * P : t0 * P + nt * P]
                    gidx[f"gidx{g}_in"] = _wrap16(seg, nt * P)
                    t0 += nt
        else:
            T_d = chunks[0][1]
            dst_cols.append(_pack_cols(ed, T_d, P, np.int32))
            gidx["eidx_in"] = _pack_cols(es, T_d, 0, np.int32)
        self_dst = np.arange(P, dtype=np.int32)
        if (k + 1) * P > item_len:
            self_dst = np.where(
                np.arange(k * P, (k + 1) * P) < item_len, self_dst, P
            ).astype(np.int32)
        dst_cols.append(self_dst[:, None])
        dst_all = np.concatenate(dst_cols, axis=1)
        S, ST = _onehot_blocks(dst_all)
        xself = np.ascontiguousarray(
            x[np.minimum(np.arange(k * P, (k + 1) * P), n_nodes - 1)]
        )
        m = {
            "x_in": x,
            "xself_in": xself,
            "x0_in": x0,
            "s_in": S,
            "st_in": ST,
            "ws_in": ws_r,
            "wd_in": wd_r,
            "w_in": w_bf,
            "bias_in": bias_r,
            "ident_in": ident,
            "beidx_in": b_eidx,
        }
        m.update(gidx)
        in_maps.append(m)
    return nc, in_maps, item_len


def assemble(results, item_len):
    y_all = np.concatenate(
        [np.asarray(results[k]["y_out"], np.float32).ravel() for k in range(N_CORES)]
    )
    return y_all[1:item_len].astype(np.float32)


def kernel(x, edge_index, W, att_src, att_dst, bias, item_len):
    from concourse import bass_utils

    nc, in_maps, item_len = prepare(
        x, edge_index, W, att_src, att_dst, bias, item_len
    )
    res = bass_utils.run_bass_kernel_spmd(nc, in_maps, core_ids=list(range(N_CORES)))
    return assemble(res.results, item_len)

